# revision 11
# baseline (speedup 1.0000x reference)
"""KroneckerLinear Trainium2 kernel (bf16 data path).

Math: out = x @ kron(f1, f2).T + bias, with x [64, 8192], f1 [128,128],
f2 [64,64], bias [8192].  Kronecker identity:
    out[b].reshape(128, 64) = f1 @ X_b @ f2.T,   X_b = x[b].reshape(128, 64)
so the 8192x8192 weight (256 MB) is never materialized; the kernel is
memory-bound on x in / out.

Sharding: batch-parallel over the 8 NeuronCores, 8 batch rows per core.

All device I/O is bf16 (tolerance is 2e-2; bf16 end-to-end error is ~5e-3):
halves DMA bytes and runs the PE at full bf16 rate instead of 1/4-rate fp32.

Per-core device program (SPMD, identical on all cores):
  input: one packed [128, 896] bf16 tensor
     cols [0:128] blk = blkdiag(f2.T, f2.T), [128:640] xt, [640:768] f1t,
     [768:896] bias_wide (bias.reshape(128, 64) tiled twice).
     xt[h*64+l, p*128+j] = x[c*8 + p + 4h, j*64+l]  (local batch lb = p+4h)
  stage 1 (apply f2), slice p = 0..3: matmul lhsT = xt_p, rhs = blk ->
     psum_v_p[j, h*64+k] = (X_{p+4h} @ f2.T)[j, k]; PSUM->SBUF bf16 copies
     alternate between ACT and DVE so they don't serialize.
  stage 2 (apply f1), slice p: matmul lhsT = f1t, rhs = v_p ->
     psum_y_p[i, h*64+k] = (f1 @ X_{p+4h} @ f2.T)[i, k]
  bias+store: DVE flat adds (+bias_wide) casting to bf16; halves stored by
     two DMAs on different HWDGE rings (SP, ACT) so descriptor-gen overlaps;
     host unpermutes.
"""

import numpy as np

N_CORES = 8
B = 64
LB = B // N_CORES  # 8 local batches per core

_CACHE = {}


def _build_nc():
    import concourse.bass as bass
    import concourse.mybir as mybir
    import concourse.tile as tile
    from concourse import bacc

    fp32 = mybir.dt.float32
    bf16 = mybir.dt.bfloat16

    nc = bacc.Bacc("TRN2", target_bir_lowering=False, debug=False)
    in_d = nc.dram_tensor("inp", [128, 896], bf16, kind="ExternalInput")
    y_d = nc.dram_tensor("y", [128, 512], bf16, kind="ExternalOutput")

    with tile.TileContext(nc) as tc:
        with (
            tc.tile_pool(name="sb", bufs=1) as sb,
            tc.tile_pool(name="psv", bufs=4, space="PSUM") as psv,
            tc.tile_pool(name="psy", bufs=4, space="PSUM") as psy,
        ):
            inp = sb.tile([128, 896], bf16)
            blk = inp[:, 0:128]
            f1t = inp[:, 640:768]
            bias_wide = inp[:, 768:896]

            def xt_slice(p):
                return inp[:, 128 + p * 128 : 256 + p * 128]

            # DMA 1: blk + xt (all stage-1 deps); DMA 2: f1t + bias.
            nc.sync.dma_start(out=inp[:, 0:640], in_=in_d[:, 0:640])
            nc.sync.dma_start(out=inp[:, 640:896], in_=in_d[:, 640:896])

            # PE warm-up: junk matmuls on a zeroed scratch tile during the
            # ~2.5us input-DMA wait, so the HAM clock throttle escalates and
            # the real matmuls run at full rate.  Results are never read.
            warm = sb.tile([128, 512], bf16)
            nc.gpsimd.memset(warm[:], 0)
            # allocated from psy: its bank is recycled by the last psum_y
            # tile (PE-to-PE WAR, program order, no stall)
            psum_w = psy.tile([128, 512], fp32, tag="psum_y")
            for _ in range(4):
                nc.tensor.matmul(
                    psum_w[:], warm[:, 0:128], warm[:], start=True, stop=True
                )

            v_sb = sb.tile([128, 512], bf16)
            out_sb = sb.tile([128, 512], bf16)

            for p in range(4):
                psum_v = psv.tile([128, 128], fp32, tag="psum_v")
                nc.tensor.matmul(
                    psum_v[:], xt_slice(p), blk, start=True, stop=True
                )
                # split the PSUM->SBUF copies across ACT and DVE so they
                # don't serialize on one engine (they gate stage 2)
                if p % 2 == 0:
                    nc.scalar.copy(v_sb[:, p * 128 : (p + 1) * 128], psum_v[:])
                else:
                    nc.vector.tensor_copy(v_sb[:, p * 128 : (p + 1) * 128], psum_v[:])

            for p in range(4):
                psum_y = psy.tile([128, 128], fp32, tag="psum_y")
                nc.tensor.matmul(
                    psum_y[:],
                    f1t,
                    v_sb[:, p * 128 : (p + 1) * 128],
                    start=True,
                    stop=True,
                )
                o_ap = out_sb[:, p * 128 : (p + 1) * 128]
                nc.vector.tensor_add(o_ap, psum_y[:], bias_wide)
                if p == 2:
                    # slices 0-2 store: SP's HWDGE ring
                    nc.sync.dma_start(out=y_d[:, 0:384], in_=out_sb[:, 0:384])
                elif p == 3:
                    # last slice alone: ACT's HWDGE ring (small + parallel gen,
                    # so the final receipt comes back as early as possible)
                    nc.scalar.dma_start(out=y_d[:, 384:512], in_=out_sb[:, 384:512])

    nc.compile()
    return nc


def _prep_core_inputs(x, factor1, factor2, bias):
    """Host-side layout prep. Returns list of per-core in_maps."""
    import ml_dtypes

    bf16 = ml_dtypes.bfloat16
    x = np.ascontiguousarray(np.asarray(x, dtype=np.float32))
    f1 = np.asarray(factor1, dtype=np.float32)
    f2 = np.asarray(factor2, dtype=np.float32)
    bias = np.asarray(bias, dtype=np.float32)

    # x -> per-core xt [128, 512]: xt[h*64+l, p*128+j] = x[c*8 + p + 4h, j*64+l]
    xc = x.reshape(N_CORES, LB, 128, 64)  # [c, lb, j, l]
    arr = xc.transpose(0, 3, 1, 2).reshape(N_CORES, 64, 2, 4, 128)  # [c, l, h, p, j]
    xt_all = arr.transpose(0, 2, 1, 3, 4).reshape(N_CORES, 128, 512)  # [c, (h,l), (p,j)]

    # packed input [128, 896] bf16: blk | xt | f1t | bias_wide
    inp_all = np.zeros((N_CORES, 128, 896), dtype=np.float32)
    inp_all[:, :64, 0:64] = f2.T
    inp_all[:, 64:, 64:128] = f2.T
    inp_all[:, :, 128:640] = xt_all
    inp_all[:, :, 640:768] = f1.T
    biasr = bias.reshape(128, 64)
    inp_all[:, :, 768:832] = biasr
    inp_all[:, :, 832:896] = biasr
    inp_all = inp_all.astype(bf16)

    return [{"inp": np.ascontiguousarray(inp_all[c])} for c in range(N_CORES)]


def kernel(x, factor1, factor2, bias):
    from concourse.bass_utils import run_bass_kernel_spmd

    if "nc" not in _CACHE:
        _CACHE["nc"] = _build_nc()
    nc = _CACHE["nc"]

    in_maps = _prep_core_inputs(x, factor1, factor2, bias)
    res = run_bass_kernel_spmd(nc, in_maps, core_ids=list(range(N_CORES)))
    kernel.last_results = res

    # y[i, p*128 + h*64 + k] = out[c*8 + p + 4h, i*64 + k]
    outs = []
    for c in range(N_CORES):
        y = np.asarray(res.results[c]["y"], dtype=np.float32)
        yr = y.reshape(128, 4, 2, 64).transpose(2, 1, 0, 3).reshape(LB, 8192)
        outs.append(yr)
    return np.concatenate(outs, axis=0)


# revision 12
# speedup vs baseline: 1.0261x; 1.0261x over previous
"""KroneckerLinear Trainium2 kernel (bf16 compute, fp32-packed DMA).

Math: out = x @ kron(f1, f2).T + bias, with x [64, 8192], f1 [128,128],
f2 [64,64], bias [8192].  Kronecker identity:
    out[b].reshape(128, 64) = f1 @ X_b @ f2.T,   X_b = x[b].reshape(128, 64)
so the 8192x8192 weight (256 MB) is never materialized; the kernel is
memory-bound on x in / out.

Sharding: batch-parallel over the 8 NeuronCores, 8 batch rows per core.

Compute is bf16 (tolerance is 2e-2; bf16 end-to-end error is ~5e-3): full-rate
PE instead of 1/4-rate fp32.  All DMA transfers are declared float32 and
bitcast to bf16 in SBUF: SDMA throughput at this size is element-rate-bound,
so moving the same bytes as half as many 4-byte elements halves drain time.

Per-core device program (SPMD, identical on all cores):
  input: one packed [128, 896]-bf16 image, moved as [128, 448] fp32
     cols (bf16) [0:128] blk = blkdiag(f2.T, f2.T), [128:640] xt,
     [640:768] f1t, [768:896] bias_wide (bias.reshape(128,64) tiled twice).
     xt[h*64+l, p*128+j] = x[c*8 + p + 4h, j*64+l]  (local batch lb = p+4h)
  stage 1 (apply f2), slice p = 0..3: matmul lhsT = xt_p, rhs = blk ->
     psum_v_p[j, h*64+k] = (X_{p+4h} @ f2.T)[j, k]; PSUM->SBUF bf16 copies
     alternate between ACT and DVE so they don't serialize.
  stage 2 (apply f1), slice p: matmul lhsT = f1t, rhs = v_p ->
     psum_y_p[i, h*64+k] = (f1 @ X_{p+4h} @ f2.T)[i, k]
  bias+store: DVE flat adds (+bias_wide) casting to bf16; halves stored by
     two DMAs on different HWDGE rings (SP, ACT) so descriptor-gen overlaps;
     host unpermutes.
"""

import numpy as np

N_CORES = 8
B = 64
LB = B // N_CORES  # 8 local batches per core

_CACHE = {}


def _build_nc():
    import concourse.bass as bass
    import concourse.mybir as mybir
    import concourse.tile as tile
    from concourse import bacc

    fp32 = mybir.dt.float32
    bf16 = mybir.dt.bfloat16

    nc = bacc.Bacc("TRN2", target_bir_lowering=False, debug=False)
    # fp32-typed views of the bf16 payload (element-rate packing)
    in_d = nc.dram_tensor("inp", [128, 448], fp32, kind="ExternalInput")
    y_d = nc.dram_tensor("y", [128, 256], fp32, kind="ExternalOutput")

    with tile.TileContext(nc) as tc:
        with (
            tc.tile_pool(name="sb", bufs=1) as sb,
            tc.tile_pool(name="psv", bufs=4, space="PSUM") as psv,
            tc.tile_pool(name="psy", bufs=4, space="PSUM") as psy,
        ):
            inp = sb.tile([128, 896], bf16)
            blk = inp[:, 0:128]
            f1t = inp[:, 640:768]
            bias_wide = inp[:, 768:896]

            def xt_slice(p):
                return inp[:, 128 + p * 128 : 256 + p * 128]

            # DMA 1: blk + xt (all stage-1 deps); DMA 2: f1t + bias.
            nc.sync.dma_start(
                out=inp[:, 0:640].bitcast(fp32), in_=in_d[:, 0:320]
            )
            nc.sync.dma_start(
                out=inp[:, 640:896].bitcast(fp32), in_=in_d[:, 320:448]
            )

            v_sb = sb.tile([128, 512], bf16)
            out_sb = sb.tile([128, 512], bf16)

            for p in range(4):
                psum_v = psv.tile([128, 128], fp32, tag="psum_v")
                nc.tensor.matmul(
                    psum_v[:], xt_slice(p), blk, start=True, stop=True
                )
                # split the PSUM->SBUF copies across ACT and DVE so they
                # don't serialize on one engine (they gate stage 2)
                if p % 2 == 0:
                    nc.scalar.copy(v_sb[:, p * 128 : (p + 1) * 128], psum_v[:])
                else:
                    nc.vector.tensor_copy(v_sb[:, p * 128 : (p + 1) * 128], psum_v[:])

            for p in range(4):
                psum_y = psy.tile([128, 128], fp32, tag="psum_y")
                nc.tensor.matmul(
                    psum_y[:],
                    f1t,
                    v_sb[:, p * 128 : (p + 1) * 128],
                    start=True,
                    stop=True,
                )
                o_ap = out_sb[:, p * 128 : (p + 1) * 128]
                nc.vector.tensor_add(o_ap, psum_y[:], bias_wide)
                if p == 1:
                    # first half store: SP's HWDGE ring
                    nc.sync.dma_start(
                        out=y_d[:, 0:128], in_=out_sb[:, 0:256].bitcast(fp32)
                    )
                elif p == 3:
                    # second half store: ACT's HWDGE ring (parallel desc-gen)
                    nc.scalar.dma_start(
                        out=y_d[:, 128:256], in_=out_sb[:, 256:512].bitcast(fp32)
                    )

    nc.compile()
    return nc


def _prep_core_inputs(x, factor1, factor2, bias):
    """Host-side layout prep. Returns list of per-core in_maps."""
    import ml_dtypes

    bf16 = ml_dtypes.bfloat16
    x = np.ascontiguousarray(np.asarray(x, dtype=np.float32))
    f1 = np.asarray(factor1, dtype=np.float32)
    f2 = np.asarray(factor2, dtype=np.float32)
    bias = np.asarray(bias, dtype=np.float32)

    # x -> per-core xt [128, 512]: xt[h*64+l, p*128+j] = x[c*8 + p + 4h, j*64+l]
    xc = x.reshape(N_CORES, LB, 128, 64)  # [c, lb, j, l]
    arr = xc.transpose(0, 3, 1, 2).reshape(N_CORES, 64, 2, 4, 128)  # [c, l, h, p, j]
    xt_all = arr.transpose(0, 2, 1, 3, 4).reshape(N_CORES, 128, 512)  # [c, (h,l), (p,j)]

    # packed input [128, 896] bf16: blk | xt | f1t | bias_wide
    inp_all = np.zeros((N_CORES, 128, 896), dtype=np.float32)
    inp_all[:, :64, 0:64] = f2.T
    inp_all[:, 64:, 64:128] = f2.T
    inp_all[:, :, 128:640] = xt_all
    inp_all[:, :, 640:768] = f1.T
    biasr = bias.reshape(128, 64)
    inp_all[:, :, 768:832] = biasr
    inp_all[:, :, 832:896] = biasr
    inp_all = np.ascontiguousarray(inp_all.astype(bf16))
    # reinterpret the bf16 payload as fp32 elements for the DMA declaration
    inp_packed = inp_all.view(np.float32).reshape(N_CORES, 128, 448)

    return [{"inp": np.ascontiguousarray(inp_packed[c])} for c in range(N_CORES)]


def kernel(x, factor1, factor2, bias):
    from concourse.bass_utils import run_bass_kernel_spmd

    if "nc" not in _CACHE:
        _CACHE["nc"] = _build_nc()
    nc = _CACHE["nc"]

    in_maps = _prep_core_inputs(x, factor1, factor2, bias)
    res = run_bass_kernel_spmd(nc, in_maps, core_ids=list(range(N_CORES)))
    kernel.last_results = res

    import ml_dtypes

    # y (fp32-packed) -> bf16 [128, 512]; y[i, p*128 + h*64 + k] =
    # out[c*8 + p + 4h, i*64 + k]
    outs = []
    for c in range(N_CORES):
        y32 = np.ascontiguousarray(np.asarray(res.results[c]["y"]))
        y = y32.view(ml_dtypes.bfloat16).astype(np.float32).reshape(128, 512)
        yr = y.reshape(128, 4, 2, 64).transpose(2, 1, 0, 3).reshape(LB, 8192)
        outs.append(yr)
    return np.concatenate(outs, axis=0)
